# revision 20
# baseline (speedup 1.0000x reference)
"""Multi-head causal self-attention (B=2, T=2048, D=1024, H=16) on 8 trn2 cores.

Sharding: data-parallel over batch (cores 0-3 -> batch 0, 4-7 -> batch 1),
tensor-parallel over heads within each 4-core group (4 heads per core).
Wq/Wk/Wv column-sharded, Wo row-sharded; each core emits its partial output
projection and the host sums the 4 partials per batch (TP unshard).

Single fused pipeline (bf16 matmul operands, fp32 PSUM accumulation):
all projection / output-projection matmuls are expressed as a stream of
fine-grained work units interleaved between attention key-tile slots, so
the tensor engine never idles behind the scalar engine's exp stream
(~1us per [128,1024] tile, the attention pace-setter).
  - scores pair per 128-key tile via 64x64 tile_position packing (2 heads
    concurrently); diagonal tiles restrict to live query columns
  - exp on scalar engine psum->sbuf bf16; causal masks built on-device
    (gpsimd affine_select) and applied on DVE
  - A@V accumulates [65,512] psum (row 64 = ones -> softmax denominators)
  - normalization: accumulators staged to sbuf immediately (frees psum),
    per-query reciprocal rows broadcast across partitions via tiny K=1
    matmuls (no DRAM bounce), DVE mul into per-(qb,hp) attT tiles
  - output projection for block qb streams as filler during block qb+1
"""

import sys

for _p in ("/opt/trn_rl_repo", "/root/.axon_site/_ro/trn_rl_repo"):
    if _p not in sys.path:
        sys.path.append(_p)

import ml_dtypes
import numpy as np

import concourse.bass as bass
import concourse.mybir as mybir
import concourse.tile as tile
from concourse import bacc
from concourse.bass_utils import run_bass_kernel_spmd

F32 = mybir.dt.float32
BF16 = mybir.dt.bfloat16

B, T, D = 2, 2048, 1024
H, DH = 16, 64
HPC = 4          # heads per core
FPC = HPC * DH   # feature dims per core (256)
NKT = T // 128   # 16 key tiles / token tiles
NQB = T // 512   # 4 query blocks
VW = DH + 1      # v width incl ones column (65)

_CACHE = {}


def _build():
    nc = bacc.Bacc("TRN2", target_bir_lowering=False, debug=False, num_devices=8)

    xt_d = nc.dram_tensor("xt", [D, T], BF16, kind="ExternalInput").ap()
    wq_d = nc.dram_tensor("wq_t", [128, 8 * FPC], BF16, kind="ExternalInput").ap()
    wk_d = nc.dram_tensor("wk_t", [128, 8 * FPC], BF16, kind="ExternalInput").ap()
    wv_d = nc.dram_tensor("wv_t", [128, 8 * FPC], BF16, kind="ExternalInput").ap()
    wo_d = nc.dram_tensor("wo_t", [128, 2 * D], BF16, kind="ExternalInput").ap()
    out_d = nc.dram_tensor("po", [T, D], BF16, kind="ExternalOutput").ap()

    with tile.TileContext(nc) as tc:
        with (
            tc.tile_pool(name="sb", bufs=1) as sb,
            tc.tile_pool(name="wk", bufs=1) as wk,
            tc.tile_pool(name="ps", bufs=1, space="PSUM") as ps,
        ):
            wq_sb = sb.tile([128, 8 * FPC], BF16)
            wk_sb = sb.tile([128, 8 * FPC], BF16)
            wv_sb = sb.tile([128, 8 * FPC], BF16)
            wo_sb = sb.tile([128, 2 * D], BF16)
            masks_sb = sb.tile([128, 4 * 1024], BF16)
            warm_sb = sb.tile([1, 16], F32)
            ones64_sb = sb.tile([1, 64], BF16)
            qT_sb = sb.tile([128, 2 * T], BF16)   # head-pair hp at cols hp*T
            kT_sb = sb.tile([128, 2 * T], BF16)
            v_sb = sb.tile([128, NKT * HPC * VW], BF16)
            att = {}
            for qb in range(NQB):
                for hp in range(2):
                    att[(qb, hp)] = sb.tile(
                        [128, 512], BF16, tag=f"att{qb}_{hp}", name=f"att{qb}_{hp}"
                    )
            xT = [
                sb.tile([128, T], BF16, tag=f"xT{kc}", name=f"xT{kc}")
                for kc in range(8)
            ]

            # upfront DMAs, priority order: first-proj deps first
            nc.sync.dma_start(wq_sb[:, 0 : 2 * FPC], wq_d[:, 0 : 2 * FPC])
            for kc in range(2):
                nc.sync.dma_start(
                    xT[kc][:, 0:512], xt_d[kc * 128 : (kc + 1) * 128, 0:512]
                )
            nc.sync.dma_start(wq_sb[:, 2 * FPC :], wq_d[:, 2 * FPC :])
            for kc in range(2, 8):
                nc.sync.dma_start(
                    xT[kc][:, 0:512], xt_d[kc * 128 : (kc + 1) * 128, 0:512]
                )
            nc.sync.dma_start(wk_sb[:], wk_d)
            nc.sync.dma_start(wv_sb[:], wv_d)
            for tb in range(1, 4):
                for kc in range(8):
                    nc.sync.dma_start(
                        xT[kc][:, tb * 512 : (tb + 1) * 512],
                        xt_d[kc * 128 : (kc + 1) * 128, tb * 512 : (tb + 1) * 512],
                    )
            nc.sync.dma_start(wo_sb[:], wo_d)

            # on-device constants (gpsimd+vector are idle during projections)
            nc.gpsimd.memset(
                v_sb[:].rearrange("p (a b) -> p a b", b=VW)[:, :, 64], 1.0
            )
            nc.gpsimd.memset(masks_sb[:], 1.0)
            for r in range(4):
                # region r: keep (1.0) where query >= key_part + 128*r, else 0
                nc.gpsimd.affine_select(
                    out=masks_sb[:, r * 1024 : (r + 1) * 1024].rearrange(
                        "p (a b) -> p a b", b=512
                    ),
                    in_=masks_sb[:, r * 1024 : (r + 1) * 1024].rearrange(
                        "p (a b) -> p a b", b=512
                    ),
                    compare_op=mybir.AluOpType.is_ge,
                    fill=0.0,
                    base=-128 * r,
                    pattern=[[0, 2], [1, 512]],
                    channel_multiplier=-1,
                )
            nc.vector.memset(ones64_sb[:], 1.0)
            nc.vector.memset(warm_sb[:], 0.0)
            # preload exp table while projections run
            nc.scalar.activation(
                warm_sb[:], warm_sb[:], mybir.ActivationFunctionType.Exp
            )

            # ---- proj / outproj work units (generators yielding thunks) ----
            def u_qproj(w_sb, dst_sb, hp, tb):
                # one [128,512] psum: 8 accumulating matmuls + 1 cast
                f = ps.tile([128, 512], F32, tag="fill", bufs=2)
                for kc in range(8):
                    yield lambda kc=kc: nc.tensor.matmul(
                        f[:],
                        w_sb[:, kc * FPC + hp * 128 : kc * FPC + (hp + 1) * 128],
                        xT[kc][:, tb * 512 : (tb + 1) * 512],
                        start=(kc == 0), stop=(kc == 7),
                    )
                yield lambda: nc.vector.tensor_copy(
                    dst_sb[:, hp * T + tb * 512 : hp * T + (tb + 1) * 512], f[:]
                )

            def u_vproj(p):
                # token tiles 2p, 2p+1 -> one [128,512] psum (16 matmuls) + cast
                f = ps.tile([128, 512], F32, tag="fill", bufs=2)
                for i in range(2):
                    tt = 2 * p + i
                    for kc in range(8):
                        yield lambda i=i, tt=tt, kc=kc: nc.tensor.matmul(
                            f[:, i * 256 : (i + 1) * 256],
                            xT[kc][:, tt * 128 : (tt + 1) * 128],
                            wv_sb[:, kc * FPC : (kc + 1) * FPC],
                            start=(kc == 0), stop=(kc == 7),
                        )
                yield lambda p=p: nc.vector.tensor_copy(
                    v_sb[:].rearrange("p (a b) -> p a b", b=VW)[
                        :, 2 * p * HPC : (2 * p + 2) * HPC, 0:DH
                    ],
                    f[:].rearrange("p (a b) -> p a b", b=DH),
                )

            def u_outproj(tt, o_t):
                # one token tile: 2 ncks x (2 accum matmuls + cast), then DMA out
                qb, tl = divmod(tt, 4)
                for nck in range(2):
                    f = ps.tile([128, 512], F32, tag="fill", bufs=2)
                    for hp in range(2):
                        yield lambda f=f, hp=hp, nck=nck: nc.tensor.matmul(
                            f[:],
                            att[(qb, hp)][:, tl * 128 : (tl + 1) * 128],
                            wo_sb[:, hp * D + nck * 512 : hp * D + (nck + 1) * 512],
                            start=(hp == 0), stop=(hp == 1),
                        )
                    yield lambda f=f, nck=nck: nc.vector.tensor_copy(
                        o_t[:, nck * 512 : (nck + 1) * 512], f[:]
                    )
                yield lambda: nc.gpsimd.dma_start(
                    out_d[tt * 128 : (tt + 1) * 128, :], o_t[:]
                )

            work = []  # flat list of thunks, consumed as filler / flushed

            def push(gen):
                work.extend(gen)

            wpos = [0]

            def pull(n):
                k = min(n, len(work) - wpos[0])
                for _ in range(k):
                    work[wpos[0]]()
                    wpos[0] += 1

            def flush():
                pull(len(work) - wpos[0])

            # ---- attention block ----
            normq = []

            def attn(qb, hp):
                while normq:  # deferred norm work: enters FIFO post-flush
                    push(normq.pop(0))
                hA, hB = 2 * hp, 2 * hp + 1
                oA = ps.tile([VW, 512], F32, tag="oA", bufs=1)
                oB = ps.tile([VW, 512], F32, tag="oB", bufs=1)
                nkt = 4 * (qb + 1)

                def attv(e, kt, off):
                    nc.tensor.matmul(
                        oA[:, off:512],
                        v_sb[:, (kt * HPC + hA) * VW : (kt * HPC + hA + 1) * VW],
                        e[:, off:512],
                        start=(kt == 0), stop=(kt == nkt - 1),
                    )
                    nc.tensor.matmul(
                        oB[:, off:512],
                        v_sb[:, (kt * HPC + hB) * VW : (kt * HPC + hB + 1) * VW],
                        e[:, 512 + off : 1024],
                        start=(kt == 0), stop=(kt == nkt - 1),
                    )

                pend = []
                for kt in range(nkt):
                    pull(2)
                    r = kt - 4 * qb
                    off = 128 * r if r > 0 else 0
                    sAB = ps.tile([128, 1024], F32, tag="sAB", bufs=2)
                    nc.tensor.matmul(
                        sAB[:, off:512],
                        kT_sb[0:64, hp * T + kt * 128 : hp * T + (kt + 1) * 128],
                        qT_sb[0:64, hp * T + qb * 512 + off : hp * T + (qb + 1) * 512],
                        start=True, stop=True, tile_position=(0, 0),
                    )
                    nc.tensor.matmul(
                        sAB[:, 512 + off : 1024],
                        kT_sb[64:128, hp * T + kt * 128 : hp * T + (kt + 1) * 128],
                        qT_sb[64:128, hp * T + qb * 512 + off : hp * T + (qb + 1) * 512],
                        start=True, stop=True, tile_position=(64, 0),
                    )
                    eAB = wk.tile([128, 1024], BF16, tag="eAB", bufs=4)
                    nc.scalar.activation(
                        eAB[:, off:1024], sAB[:, off:1024],
                        mybir.ActivationFunctionType.Exp, scale=0.125,
                    )
                    if r >= 0:  # diagonal tile: mask k > q
                        nc.vector.tensor_mul(
                            eAB[:, off:1024], eAB[:, off:1024],
                            masks_sb[:, r * 1024 + off : (r + 1) * 1024],
                        )
                    pend.append((eAB, kt, off))
                    if len(pend) > 2:
                        attv(*pend.pop(0))
                while pend:
                    pull(2)
                    attv(*pend.pop(0))
                # stage accumulators + denom rows to sbuf (frees psum)
                stgA = wk.tile([64, 512], F32, tag="stgA", bufs=2)
                nc.vector.tensor_copy(stgA[:], oA[0:64, :])
                stgB = wk.tile([64, 512], F32, tag="stgB", bufs=2)
                nc.vector.tensor_copy(stgB[:], oB[0:64, :])
                dn = wk.tile([1, 1024], F32, tag="dn", bufs=2)
                nc.scalar.copy(dn[0:1, 0:512], oA[64:65, :])
                nc.scalar.copy(dn[0:1, 512:1024], oB[64:65, :])
                # lane-pack, reciprocal, unpack to a bf16 row pair
                packed = wk.tile([128, 8], F32, tag="packed", bufs=2)
                nc.sync.dma_start(
                    packed[:], dn[:].rearrange("r (g e) -> r g e", e=8)
                )
                rpacked = wk.tile([128, 8], BF16, tag="rpacked", bufs=2)
                with nc.allow_low_precision(reason="bf16 recip rows for bcast"):
                    nc.vector.reciprocal(rpacked[:], packed[:])
                rr = wk.tile([1, 1024], BF16, tag="rr", bufs=2)
                nc.sync.dma_start(
                    rr[:].rearrange("r (g e) -> r g e", e=8), rpacked[:]
                )
                # broadcast recip rows via K=1 matmuls + normalize, as filler
                # units so the chain latency hides under the next block
                att_t = att[(qb, hp)]

                def u_norm(rr=rr, stgA=stgA, stgB=stgB, att_t=att_t):
                    bcA = ps.tile([128, 512], F32, tag="fill", bufs=2)
                    yield lambda: nc.tensor.matmul(
                        bcA[0:64, :], ones64_sb[:], rr[0:1, 0:512],
                        start=True, stop=True,
                    )
                    bcB = ps.tile([128, 512], F32, tag="fill", bufs=2)
                    yield lambda: nc.tensor.matmul(
                        bcB[0:64, :], ones64_sb[:], rr[0:1, 512:1024],
                        start=True, stop=True,
                    )
                    yield lambda: nc.vector.tensor_mul(
                        att_t[0:64, :], stgA[:], bcA[0:64, :]
                    )
                    yield lambda: nc.vector.tensor_mul(
                        att_t[64:128, :], stgB[:], bcB[0:64, :]
                    )

                normq.append(u_norm())

            def push_outproj(qb):
                for t4 in range(4):
                    tt = qb * 4 + t4
                    o_t = wk.tile([128, D], BF16, tag="osb", bufs=3)
                    push(u_outproj(tt, o_t))

            # ---- fused schedule ----
            push(u_qproj(wq_sb, qT_sb, 0, 0))
            push(u_qproj(wk_sb, kT_sb, 0, 0))
            push(u_vproj(0))
            push(u_vproj(1))
            flush()
            attn(0, 0)
            push(u_qproj(wq_sb, qT_sb, 1, 0))
            push(u_qproj(wk_sb, kT_sb, 1, 0))
            flush()
            attn(0, 1)
            push(u_qproj(wq_sb, qT_sb, 0, 1))
            push(u_qproj(wk_sb, kT_sb, 0, 1))
            push(u_vproj(2))
            push(u_vproj(3))
            flush()
            attn(1, 0)
            push(u_qproj(wq_sb, qT_sb, 1, 1))
            push(u_qproj(wk_sb, kT_sb, 1, 1))
            push_outproj(0)
            flush()
            attn(1, 1)
            push(u_qproj(wq_sb, qT_sb, 0, 2))
            push(u_qproj(wk_sb, kT_sb, 0, 2))
            push(u_vproj(4))
            push(u_vproj(5))
            flush()
            attn(2, 0)
            push(u_qproj(wq_sb, qT_sb, 1, 2))
            push(u_qproj(wk_sb, kT_sb, 1, 2))
            push_outproj(1)
            flush()
            attn(2, 1)
            push(u_qproj(wq_sb, qT_sb, 0, 3))
            push(u_qproj(wk_sb, kT_sb, 0, 3))
            push(u_vproj(6))
            push(u_vproj(7))
            flush()
            attn(3, 0)
            push(u_qproj(wq_sb, qT_sb, 1, 3))
            push(u_qproj(wk_sb, kT_sb, 1, 3))
            push_outproj(2)
            flush()
            attn(3, 1)
            while normq:
                push(normq.pop(0))
            push_outproj(3)
            flush()

    nc.compile()
    return nc


def _prepack(w, bf):
    # [c*128, f] -> [128, c*f] (SBUF chunk layout)
    c = w.shape[0] // 128
    return np.ascontiguousarray(
        w.reshape(c, 128, w.shape[1]).transpose(1, 0, 2).reshape(128, -1)
    ).astype(bf)


def _prep_in_maps(x, Wq, Wk, Wv, Wo):
    x = np.asarray(x, dtype=np.float32)
    bf = ml_dtypes.bfloat16
    Wq = np.asarray(Wq, dtype=np.float32)
    Wk = np.asarray(Wk, dtype=np.float32)
    Wv = np.asarray(Wv, dtype=np.float32)
    Wo = np.asarray(Wo, dtype=np.float32)
    in_maps = []
    for c in range(8):
        b, g = divmod(c, 4)
        sl = slice(g * FPC, (g + 1) * FPC)
        in_maps.append(
            {
                "xt": np.ascontiguousarray(x[b].T).astype(bf),
                "wq_t": _prepack(Wq[sl, :].T, bf),
                "wk_t": _prepack(Wk[sl, :].T, bf),
                "wv_t": _prepack(Wv[sl, :].T, bf),
                "wo_t": _prepack(Wo[:, sl].T, bf),
            }
        )
    return in_maps


def _get_nc():
    if "nc" not in _CACHE:
        _CACHE["nc"] = _build()
    return _CACHE["nc"]


def _assemble(results):
    out = np.empty((B, T, D), dtype=np.float32)
    for b in range(B):
        out[b] = (
            results[4 * b]["po"].astype(np.float32)
            + results[4 * b + 1]["po"].astype(np.float32)
            + results[4 * b + 2]["po"].astype(np.float32)
            + results[4 * b + 3]["po"].astype(np.float32)
        )
    return out


def kernel(x, Wq, Wk, Wv, Wo):
    nc = _get_nc()
    in_maps = _prep_in_maps(x, Wq, Wk, Wv, Wo)
    res = run_bass_kernel_spmd(nc, in_maps, core_ids=list(range(8)))
    return _assemble(res.results)


def kernel_with_trace(x, Wq, Wk, Wv, Wo, **kw):
    nc = _get_nc()
    in_maps = _prep_in_maps(x, Wq, Wk, Wv, Wo)
    res = run_bass_kernel_spmd(nc, in_maps, core_ids=list(range(8)), trace=True, **kw)
    return _assemble(res.results), res


# revision 22
# speedup vs baseline: 1.0887x; 1.0887x over previous
"""Multi-head causal self-attention (B=2, T=2048, D=1024, H=16) on 8 trn2 cores.

Sharding: data-parallel over batch (cores 0-3 -> batch 0, 4-7 -> batch 1),
tensor-parallel over heads within each 4-core group (4 heads per core).
Wq/Wk/Wv column-sharded, Wo row-sharded; each core emits its partial output
projection and the host sums the 4 partials per batch (TP unshard).

Single fused pipeline (bf16 matmul operands, fp32 PSUM accumulation):
all projection / output-projection matmuls are expressed as a stream of
fine-grained work units interleaved between attention key-tile slots, so
the tensor engine never idles behind the scalar engine's exp stream
(~1us per [128,1024] tile, the attention pace-setter).
  - scores pair per 128-key tile via 64x64 tile_position packing (2 heads
    concurrently); diagonal tiles restrict to live query columns
  - exp on scalar engine psum->sbuf bf16; causal masks built on-device
    (gpsimd affine_select) and applied on DVE
  - A@V accumulates [65,512] psum (row 64 = ones -> softmax denominators)
  - normalization: accumulators staged to sbuf immediately (frees psum),
    per-query reciprocal rows broadcast across partitions via tiny K=1
    matmuls (no DRAM bounce), DVE mul into per-(qb,hp) attT tiles
  - output projection for block qb streams as filler during block qb+1
"""

import sys

for _p in ("/opt/trn_rl_repo", "/root/.axon_site/_ro/trn_rl_repo"):
    if _p not in sys.path:
        sys.path.append(_p)

import ml_dtypes
import numpy as np

import concourse.bass as bass
import concourse.mybir as mybir
import concourse.tile as tile
from concourse import bacc
from concourse.bass_utils import run_bass_kernel_spmd

F32 = mybir.dt.float32
BF16 = mybir.dt.bfloat16

B, T, D = 2, 2048, 1024
H, DH = 16, 64
HPC = 4          # heads per core
FPC = HPC * DH   # feature dims per core (256)
NKT = T // 128   # 16 key tiles / token tiles
NQB = T // 512   # 4 query blocks
VW = DH + 1      # v width incl ones column (65)

_CACHE = {}


def _build():
    nc = bacc.Bacc("TRN2", target_bir_lowering=False, debug=False, num_devices=8)

    xt_d = nc.dram_tensor("xt", [D, T], BF16, kind="ExternalInput").ap()
    wq_d = nc.dram_tensor("wq_t", [128, 8 * FPC], BF16, kind="ExternalInput").ap()
    wk_d = nc.dram_tensor("wk_t", [128, 8 * FPC], BF16, kind="ExternalInput").ap()
    wv_d = nc.dram_tensor("wv_t", [128, 8 * FPC], BF16, kind="ExternalInput").ap()
    wo_d = nc.dram_tensor("wo_t", [128, 2 * D], BF16, kind="ExternalInput").ap()
    out_d = nc.dram_tensor("po", [T, D], BF16, kind="ExternalOutput").ap()

    with tile.TileContext(nc) as tc:
        with (
            tc.tile_pool(name="sb", bufs=1) as sb,
            tc.tile_pool(name="wk", bufs=1) as wk,
            tc.tile_pool(name="ps", bufs=1, space="PSUM") as ps,
        ):
            wq_sb = sb.tile([128, 8 * FPC], BF16)
            wk_sb = sb.tile([128, 8 * FPC], BF16)
            wv_sb = sb.tile([128, 8 * FPC], BF16)
            wo_sb = sb.tile([128, 2 * D], BF16)
            masks_sb = sb.tile([128, 4 * 1024], BF16)
            warm_sb = sb.tile([1, 16], F32)
            ones64_sb = sb.tile([1, 64], BF16)
            qT_sb = sb.tile([128, 2 * T], BF16)   # head-pair hp at cols hp*T
            kT_sb = sb.tile([128, 2 * T], BF16)
            v_sb = sb.tile([128, NKT * HPC * VW], BF16)
            att = {}
            for qb in range(NQB):
                for hp in range(2):
                    att[(qb, hp)] = sb.tile(
                        [128, 512], BF16, tag=f"att{qb}_{hp}", name=f"att{qb}_{hp}"
                    )
            xT = [
                sb.tile([128, T], BF16, tag=f"xT{kc}", name=f"xT{kc}")
                for kc in range(8)
            ]

            # upfront DMAs, priority order: first-proj deps first
            nc.sync.dma_start(wq_sb[:, 0 : 2 * FPC], wq_d[:, 0 : 2 * FPC])
            for kc in range(2):
                nc.sync.dma_start(
                    xT[kc][:, 0:512], xt_d[kc * 128 : (kc + 1) * 128, 0:512]
                )
            nc.sync.dma_start(wq_sb[:, 2 * FPC :], wq_d[:, 2 * FPC :])
            for kc in range(2, 8):
                nc.sync.dma_start(
                    xT[kc][:, 0:512], xt_d[kc * 128 : (kc + 1) * 128, 0:512]
                )
            nc.sync.dma_start(wk_sb[:], wk_d)
            nc.sync.dma_start(wv_sb[:], wv_d)
            for tb in range(1, 4):
                for kc in range(8):
                    nc.sync.dma_start(
                        xT[kc][:, tb * 512 : (tb + 1) * 512],
                        xt_d[kc * 128 : (kc + 1) * 128, tb * 512 : (tb + 1) * 512],
                    )
            nc.sync.dma_start(wo_sb[:], wo_d)

            # on-device constants (gpsimd+vector are idle during projections)
            nc.gpsimd.memset(
                v_sb[:].rearrange("p (a b) -> p a b", b=VW)[:, :, 64], 1.0
            )
            nc.gpsimd.memset(masks_sb[:], 1.0)
            for r in range(4):
                # region r: keep (1.0) where query >= key_part + 128*r, else 0
                nc.gpsimd.affine_select(
                    out=masks_sb[:, r * 1024 : (r + 1) * 1024].rearrange(
                        "p (a b) -> p a b", b=512
                    ),
                    in_=masks_sb[:, r * 1024 : (r + 1) * 1024].rearrange(
                        "p (a b) -> p a b", b=512
                    ),
                    compare_op=mybir.AluOpType.is_ge,
                    fill=0.0,
                    base=-128 * r,
                    pattern=[[0, 2], [1, 512]],
                    channel_multiplier=-1,
                )
            nc.vector.memset(ones64_sb[:], 1.0)
            nc.vector.memset(warm_sb[:], 0.0)
            # preload exp table while projections run
            nc.scalar.activation(
                warm_sb[:], warm_sb[:], mybir.ActivationFunctionType.Exp
            )

            # ---- proj / outproj work units (generators yielding thunks) ----
            def u_qproj(w_sb, dst_sb, hp, tb):
                # one [128,512] psum: 8 accumulating matmuls + 1 cast
                f = ps.tile([128, 512], F32, tag="fill", bufs=2)
                for kc in range(8):
                    yield lambda kc=kc: nc.tensor.matmul(
                        f[:],
                        w_sb[:, kc * FPC + hp * 128 : kc * FPC + (hp + 1) * 128],
                        xT[kc][:, tb * 512 : (tb + 1) * 512],
                        start=(kc == 0), stop=(kc == 7),
                    )
                yield lambda: nc.vector.tensor_copy(
                    dst_sb[:, hp * T + tb * 512 : hp * T + (tb + 1) * 512], f[:]
                )

            def u_vproj(p):
                # token tiles 2p, 2p+1 -> one [128,512] psum (16 matmuls) + cast
                f = ps.tile([128, 512], F32, tag="fill", bufs=2)
                for i in range(2):
                    tt = 2 * p + i
                    for kc in range(8):
                        yield lambda i=i, tt=tt, kc=kc: nc.tensor.matmul(
                            f[:, i * 256 : (i + 1) * 256],
                            xT[kc][:, tt * 128 : (tt + 1) * 128],
                            wv_sb[:, kc * FPC : (kc + 1) * FPC],
                            start=(kc == 0), stop=(kc == 7),
                        )
                yield lambda p=p: nc.vector.tensor_copy(
                    v_sb[:].rearrange("p (a b) -> p a b", b=VW)[
                        :, 2 * p * HPC : (2 * p + 2) * HPC, 0:DH
                    ],
                    f[:].rearrange("p (a b) -> p a b", b=DH),
                )

            def u_outproj(tt, o_t):
                # one token tile: 2 ncks x (2 accum matmuls + cast), then DMA out
                qb, tl = divmod(tt, 4)
                for nck in range(2):
                    f = ps.tile([128, 512], F32, tag="fill", bufs=2)
                    for hp in range(2):
                        yield lambda f=f, hp=hp, nck=nck: nc.tensor.matmul(
                            f[:],
                            att[(qb, hp)][:, tl * 128 : (tl + 1) * 128],
                            wo_sb[:, hp * D + nck * 512 : hp * D + (nck + 1) * 512],
                            start=(hp == 0), stop=(hp == 1),
                        )
                    yield lambda f=f, nck=nck: nc.vector.tensor_copy(
                        o_t[:, nck * 512 : (nck + 1) * 512], f[:]
                    )
                yield lambda: nc.gpsimd.dma_start(
                    out_d[tt * 128 : (tt + 1) * 128, :], o_t[:]
                )

            work = []  # flat list of thunks, consumed as filler / flushed

            def push(gen):
                work.extend(gen)

            wpos = [0]

            def pull(n):
                k = min(n, len(work) - wpos[0])
                for _ in range(k):
                    work[wpos[0]]()
                    wpos[0] += 1

            def mark():
                return len(work)

            def flush_to(m):
                pull(m - wpos[0])

            def flush():
                pull(len(work) - wpos[0])

            # ---- attention block ----
            normq = []

            def attn(qb, hp):
                while normq:  # deferred norm work: enters FIFO post-flush
                    push(normq.pop(0))
                hA, hB = 2 * hp, 2 * hp + 1
                oA = ps.tile([VW, 512], F32, tag="oA", bufs=1)
                oB = ps.tile([VW, 512], F32, tag="oB", bufs=1)
                nkt = 4 * (qb + 1)

                def attv(e, kt, off):
                    nc.tensor.matmul(
                        oA[:, off:512],
                        v_sb[:, (kt * HPC + hA) * VW : (kt * HPC + hA + 1) * VW],
                        e[:, off:512],
                        start=(kt == 0), stop=(kt == nkt - 1),
                    )
                    nc.tensor.matmul(
                        oB[:, off:512],
                        v_sb[:, (kt * HPC + hB) * VW : (kt * HPC + hB + 1) * VW],
                        e[:, 512 + off : 1024],
                        start=(kt == 0), stop=(kt == nkt - 1),
                    )

                pend = []
                for kt in range(nkt):
                    pull(2)
                    r = kt - 4 * qb
                    off = 128 * r if r > 0 else 0
                    sAB = ps.tile([128, 1024], F32, tag="sAB", bufs=2)
                    nc.tensor.matmul(
                        sAB[:, off:512],
                        kT_sb[0:64, hp * T + kt * 128 : hp * T + (kt + 1) * 128],
                        qT_sb[0:64, hp * T + qb * 512 + off : hp * T + (qb + 1) * 512],
                        start=True, stop=True, tile_position=(0, 0),
                    )
                    nc.tensor.matmul(
                        sAB[:, 512 + off : 1024],
                        kT_sb[64:128, hp * T + kt * 128 : hp * T + (kt + 1) * 128],
                        qT_sb[64:128, hp * T + qb * 512 + off : hp * T + (qb + 1) * 512],
                        start=True, stop=True, tile_position=(64, 0),
                    )
                    eAB = wk.tile([128, 1024], BF16, tag="eAB", bufs=4)
                    nc.scalar.activation(
                        eAB[:, off:1024], sAB[:, off:1024],
                        mybir.ActivationFunctionType.Exp, scale=0.125,
                    )
                    if r >= 0:  # diagonal tile: mask k > q
                        nc.vector.tensor_mul(
                            eAB[:, off:1024], eAB[:, off:1024],
                            masks_sb[:, r * 1024 + off : (r + 1) * 1024],
                        )
                    pend.append((eAB, kt, off))
                    if len(pend) > 2:
                        attv(*pend.pop(0))
                while pend:
                    pull(2)
                    attv(*pend.pop(0))
                # stage accumulators + denom rows to sbuf (frees psum)
                stgA = wk.tile([64, 512], F32, tag="stgA", bufs=2)
                nc.vector.tensor_copy(stgA[:], oA[0:64, :])
                stgB = wk.tile([64, 512], F32, tag="stgB", bufs=2)
                nc.vector.tensor_copy(stgB[:], oB[0:64, :])
                dn = wk.tile([1, 1024], F32, tag="dn", bufs=2)
                nc.scalar.copy(dn[0:1, 0:512], oA[64:65, :])
                nc.scalar.copy(dn[0:1, 512:1024], oB[64:65, :])
                # lane-pack, reciprocal, unpack to a bf16 row pair
                packed = wk.tile([128, 8], F32, tag="packed", bufs=2)
                nc.sync.dma_start(
                    packed[:], dn[:].rearrange("r (g e) -> r g e", e=8)
                )
                rpacked = wk.tile([128, 8], BF16, tag="rpacked", bufs=2)
                with nc.allow_low_precision(reason="bf16 recip rows for bcast"):
                    nc.vector.reciprocal(rpacked[:], packed[:])
                rr = wk.tile([1, 1024], BF16, tag="rr", bufs=2)
                nc.sync.dma_start(
                    rr[:].rearrange("r (g e) -> r g e", e=8), rpacked[:]
                )
                # broadcast recip rows via K=1 matmuls + normalize, as filler
                # units so the chain latency hides under the next block
                att_t = att[(qb, hp)]

                def u_norm(rr=rr, stgA=stgA, stgB=stgB, att_t=att_t):
                    bcA = ps.tile([128, 512], F32, tag="fill", bufs=2)
                    yield lambda: nc.tensor.matmul(
                        bcA[0:64, :], ones64_sb[:], rr[0:1, 0:512],
                        start=True, stop=True,
                    )
                    bcB = ps.tile([128, 512], F32, tag="fill", bufs=2)
                    yield lambda: nc.tensor.matmul(
                        bcB[0:64, :], ones64_sb[:], rr[0:1, 512:1024],
                        start=True, stop=True,
                    )
                    yield lambda: nc.vector.tensor_mul(
                        att_t[0:64, :], stgA[:], bcA[0:64, :]
                    )
                    yield lambda: nc.vector.tensor_mul(
                        att_t[64:128, :], stgB[:], bcB[0:64, :]
                    )

                normq.append(u_norm())

            def push_outproj(qb):
                for t4 in range(4):
                    tt = qb * 4 + t4
                    o_t = wk.tile([128, D], BF16, tag="osb", bufs=3)
                    push(u_outproj(tt, o_t))

            # ---- fused schedule ----
            # deps for block X are flushed just before X; units for X+1/X+2
            # stay queued so every block has filler work for the tensor engine
            push(u_qproj(wq_sb, qT_sb, 0, 0))
            push(u_qproj(wk_sb, kT_sb, 0, 0))
            push(u_vproj(0))
            push(u_vproj(1))
            flush()
            push(u_qproj(wq_sb, qT_sb, 1, 0))
            push(u_qproj(wk_sb, kT_sb, 1, 0))
            m01 = mark()
            push(u_qproj(wq_sb, qT_sb, 0, 1))
            push(u_vproj(2))
            push(u_vproj(3))
            attn(0, 0)
            flush_to(m01)
            push(u_qproj(wk_sb, kT_sb, 0, 1))
            m10 = mark()
            attn(0, 1)
            push(u_qproj(wq_sb, qT_sb, 1, 1))
            push(u_qproj(wk_sb, kT_sb, 1, 1))
            m11 = mark()
            flush_to(m10)
            attn(1, 0)
            push(u_qproj(wq_sb, qT_sb, 0, 2))
            push(u_vproj(4))
            push(u_vproj(5))
            push(u_qproj(wk_sb, kT_sb, 0, 2))
            m20 = mark()
            push_outproj(0)
            flush_to(m11)
            attn(1, 1)
            push(u_qproj(wq_sb, qT_sb, 1, 2))
            push(u_qproj(wk_sb, kT_sb, 1, 2))
            m21 = mark()
            flush_to(m20)
            attn(2, 0)
            push(u_qproj(wq_sb, qT_sb, 0, 3))
            push(u_vproj(6))
            push(u_vproj(7))
            push(u_qproj(wk_sb, kT_sb, 0, 3))
            m30 = mark()
            push_outproj(1)
            flush_to(m21)
            attn(2, 1)
            push(u_qproj(wq_sb, qT_sb, 1, 3))
            push(u_qproj(wk_sb, kT_sb, 1, 3))
            m31 = mark()
            flush_to(m30)
            attn(3, 0)
            push_outproj(2)
            flush_to(m31)
            attn(3, 1)
            while normq:
                push(normq.pop(0))
            push_outproj(3)
            flush()

    nc.compile()
    return nc


def _prepack(w, bf):
    # [c*128, f] -> [128, c*f] (SBUF chunk layout)
    c = w.shape[0] // 128
    return np.ascontiguousarray(
        w.reshape(c, 128, w.shape[1]).transpose(1, 0, 2).reshape(128, -1)
    ).astype(bf)


def _prep_in_maps(x, Wq, Wk, Wv, Wo):
    x = np.asarray(x, dtype=np.float32)
    bf = ml_dtypes.bfloat16
    Wq = np.asarray(Wq, dtype=np.float32)
    Wk = np.asarray(Wk, dtype=np.float32)
    Wv = np.asarray(Wv, dtype=np.float32)
    Wo = np.asarray(Wo, dtype=np.float32)
    in_maps = []
    for c in range(8):
        b, g = divmod(c, 4)
        sl = slice(g * FPC, (g + 1) * FPC)
        in_maps.append(
            {
                "xt": np.ascontiguousarray(x[b].T).astype(bf),
                "wq_t": _prepack(Wq[sl, :].T, bf),
                "wk_t": _prepack(Wk[sl, :].T, bf),
                "wv_t": _prepack(Wv[sl, :].T, bf),
                "wo_t": _prepack(Wo[:, sl].T, bf),
            }
        )
    return in_maps


def _get_nc():
    if "nc" not in _CACHE:
        _CACHE["nc"] = _build()
    return _CACHE["nc"]


def _assemble(results):
    out = np.empty((B, T, D), dtype=np.float32)
    for b in range(B):
        out[b] = (
            results[4 * b]["po"].astype(np.float32)
            + results[4 * b + 1]["po"].astype(np.float32)
            + results[4 * b + 2]["po"].astype(np.float32)
            + results[4 * b + 3]["po"].astype(np.float32)
        )
    return out


def kernel(x, Wq, Wk, Wv, Wo):
    nc = _get_nc()
    in_maps = _prep_in_maps(x, Wq, Wk, Wv, Wo)
    res = run_bass_kernel_spmd(nc, in_maps, core_ids=list(range(8)))
    return _assemble(res.results)


def kernel_with_trace(x, Wq, Wk, Wv, Wo, **kw):
    nc = _get_nc()
    in_maps = _prep_in_maps(x, Wq, Wk, Wv, Wo)
    res = run_bass_kernel_spmd(nc, in_maps, core_ids=list(range(8)), trace=True, **kw)
    return _assemble(res.results), res
